# revision 1
# baseline (speedup 1.0000x reference)
"""CenterLoss kernel for Trainium2 (8 NeuronCores, SPMD data-parallel).

Reference computes
    distmat[b,c] = ||x_b||^2 + ||c_c||^2 - 2<x_b, c_c>          [B, C]
    loss = sum(clip(distmat * onehot(labels), 1e-12, 1e12)) / B

Only distmat[b, labels[b]] survives the mask; each of the B*(C-1) masked
zeros becomes exactly 1e-12 under the clip. So instead of the [8192, 10000]
distmat (42 GFLOP), each core gathers its rows' centers with indirect DMA
and computes per-row squared distances; the host adds the closed-form
constant B*(C-1)*1e-12 and divides by B.

Sharding: batch split 8 ways (1024 rows/core), centers replicated.

Per-core kernel (all stock ISA):
  - one [128, 8] int32 idx tile (labels, row p*8+g at [p, g])
  - one contiguous 1MB x load as [128, 8, 256] (row p*8+g at [p, g, :])
  - 8 indirect row-gathers (Q7 SWDGE, 128 rows each) whose offset APs are
    column slices of the idx tile; compute pipelined behind them:
    DVE subtract, ACT Square with accum_out giving the row reduction.
  - [128, 8] partial distances DMA'd out; host clamps at 1e-12 and sums.

Hard-won HW constraints baked in here (this runtime rejects/crashes
otherwise):
  - Use Bacc, and finalize() before run: TRN2 codegen allows ONE sync-wait
    per instruction; Bacc's generate_event_semaphores splits multi-waits,
    and the bass2jax path serializes the module without finalizing.
  - Stock instructions only: custom "Ant" ISA ops (tensor_tensor_reduce,
    dma_gather, ...) kill the exec unit (NRT_EXEC_UNIT_UNRECOVERABLE).
  - No in-place DVE ops (out aliasing an input) — same crash.
  - indirect_dma_start: offset AP may be a [128, 1] column slice, but the
    dest must be a whole [128, D] tile; multi-column offsets or strided
    dest slices gather garbage.
"""

import numpy as np

from concourse import bacc, bass, mybir
import concourse.tile as tile
from concourse.bass_utils import run_bass_kernel_spmd

B = 8192
C = 10000
D = 256
N_CORES = 8
BL = B // N_CORES  # rows per core
P = 128            # SBUF partitions
G = BL // P        # row groups per core

_CLIP_LO = 1e-12

_nc_cache = None


def _build():
    global _nc_cache
    if _nc_cache is not None:
        return _nc_cache

    nc = bacc.Bacc()
    x_l = nc.dram_tensor("x_local", [BL, D], mybir.dt.float32, kind="ExternalInput")
    lab_l = nc.dram_tensor("labels_local", [BL], mybir.dt.int32, kind="ExternalInput")
    cen = nc.dram_tensor("centers", [C, D], mybir.dt.float32, kind="ExternalInput")
    out = nc.dram_tensor("partials", [P, G], mybir.dt.float32, kind="ExternalOutput")

    with tile.TileContext(nc) as tc:
        with (
            tc.tile_pool(name="big", bufs=1) as big,
            tc.tile_pool(name="work", bufs=4) as work,
            # gather dests get all 8 slots: late gathers then never carry a
            # slot-release wait, keeping the Q7 chain free of EVSEM stalls
            tc.tile_pool(name="ctp", bufs=G) as ctp,
        ):
            lt = big.tile([P, G], mybir.dt.int32)
            xt = big.tile([P, G, D], mybir.dt.float32)
            acc = big.tile([P, G], mybir.dt.float32)

            # idx tile first: the whole gather chain hangs off it
            nc.sync.dma_start(out=lt[:], in_=lab_l[:].rearrange("(p g) -> p g", g=G))
            # x in halves so early groups aren't gated on the full 1MB
            x_ap = x_l[:].rearrange("(p g) d -> p g d", g=G)
            nc.sync.dma_start(out=xt[:, 0:G // 2, :], in_=x_ap[:, 0:G // 2, :])
            nc.sync.dma_start(out=xt[:, G // 2:, :], in_=x_ap[:, G // 2:, :])

            for g in range(G):
                ct = ctp.tile([P, D], mybir.dt.float32, tag="ct")
                nc.gpsimd.indirect_dma_start(
                    out=ct[:],
                    out_offset=None,
                    in_=cen[:],
                    in_offset=bass.IndirectOffsetOnAxis(ap=lt[:, g:g + 1], axis=0),
                )
                dt = work.tile([P, D], mybir.dt.float32, tag="dt")
                nc.vector.tensor_sub(out=dt[:], in0=xt[:, g, :], in1=ct[:])
                sq = work.tile([P, D], mybir.dt.float32, tag="sq")
                nc.scalar.activation(
                    out=sq[:],
                    in_=dt[:],
                    func=mybir.ActivationFunctionType.Square,
                    accum_out=acc[:, g:g + 1],
                )
            nc.sync.dma_start(out=out[:], in_=acc[:])

    nc.finalize()
    _nc_cache = nc
    return nc


def _run(x, labels, centers, **spmd_kwargs):
    nc = _build()
    x = np.ascontiguousarray(np.asarray(x), dtype=np.float32)
    labels = np.ascontiguousarray(np.asarray(labels)).astype(np.int32)
    centers = np.ascontiguousarray(np.asarray(centers), dtype=np.float32)

    in_maps = []
    for c in range(N_CORES):
        sl = slice(c * BL, (c + 1) * BL)
        in_maps.append(
            {
                "x_local": x[sl],
                "labels_local": labels[sl],
                "centers": centers,
            }
        )
    res = run_bass_kernel_spmd(nc, in_maps, list(range(N_CORES)), **spmd_kwargs)
    partials = np.stack([r["partials"] for r in res.results])  # [8, P, G]
    clamped = np.maximum(partials.astype(np.float64), _CLIP_LO)
    loss = (clamped.sum() + B * (C - 1) * _CLIP_LO) / B
    return np.asarray(loss, dtype=np.float32), res


def kernel(x, labels, centers):
    loss, _ = _run(x, labels, centers)
    return loss



# revision 2
# speedup vs baseline: 1.1216x; 1.1216x over previous
"""CenterLoss kernel for Trainium2 (8 NeuronCores, SPMD data-parallel).

Reference computes
    distmat[b,c] = ||x_b||^2 + ||c_c||^2 - 2<x_b, c_c>          [B, C]
    loss = sum(clip(distmat * onehot(labels), 1e-12, 1e12)) / B

Only distmat[b, labels[b]] survives the mask; each of the B*(C-1) masked
zeros becomes exactly 1e-12 under the clip. So instead of the [8192, 10000]
distmat (42 GFLOP), each core gathers its rows' centers with indirect DMA
and computes per-row squared distances; the host adds the closed-form
constant B*(C-1)*1e-12 and divides by B. Every real row distance is a
chi^2-like sum of 256 squared normals (~512 >> 1e-12), so the per-row clip
is a provable no-op and only partial SUMS leave the device.

Sharding: batch split 8 ways (1024 rows/core), centers replicated.

Per-core structure (all stock ISA):
  - labels DMA'd first on Sync as [128, 8] (row p*8+g at [p, g]).
  - x as two linear 512KB DMAs into [128, 8, 256] (partition p holds rows
    8p..8p+7 contiguously -> perfectly sequential HBM reads).
  - 8 indirect row-gathers (SWDGE) of 128 rows each, offset APs are single
    column slices of the labels tile.
  - DVE subtracts pipeline behind the gathers into one [128, 2048] diff
    tile; ACT Square runs as 3 accum chunks (groups 0-3, 4-6, 7) so the
    post-last-gather tail is only sub + one 256-col ACT + ACCUM read.
  - acc [128, 8] DMA'd out (cols 0..2 meaningful); host sums in f64.

Measured timing model (exec ~27.5us, vs 28.6us session-start baseline;
whole-trace span is the graded metric and includes ~6us framework preamble
and ~3.5us teardown):
  ~6.1-6.8  framework preamble (sem-table init, engine barriers) — FIXED,
            independent of kernel instruction count.
  +0.7      labels DMA issue (Sync HWDGE) + 1.7us land latency
            (DGE delay 650ns + SEM_PROP 900ns).
  ~9.5-20.7 gather chain: 8 x DMA_INDIRECT at ~1410ns pitch.
  +1.8      last gather data lands (descriptor drain + sem propagation)
  +1.2      sub7 + 256-col Square/accum + ACCUM read
  +0.6+1.7  out DMA issue + completion leg; +~1.9us profiler tail.

Hard-won HW constraints baked in (this runtime rejects/crashes or
silently corrupts otherwise):
  - Use Bacc, and finalize() before run: TRN2 codegen allows ONE sync-wait
    per instruction; Bacc's generate_event_semaphores splits multi-waits.
  - Stock instructions only: custom "Ant" ISA ops (tensor_tensor_reduce,
    dma_gather, ...) kill the exec unit (NRT_EXEC_UNIT_UNRECOVERABLE).
  - No in-place DVE ops (out aliasing an input) — same crash.
  - indirect_dma_start: offset AP must be a [128, 1] column (one offset per
    partition). Multi-column offset APs are BROKEN in the SWDGE ucode
    (probed: first descriptor reuses the previous gather's stale offset
    address, in-partition descriptors re-read one column, partition
    advance resets to column 0, some descriptors never fire,
    nondeterministic across cores). Offsets must be in SBUF (walrus
    rejects DRAM: "Vector-dynamic-offsets location must be SB").
  - DMA_INDIRECT costs 994ns fixed SWDGE overhead + 0.34ns/descriptor,
    pitch ~1410ns back-to-back, independent of Tile framework glue
    (probed with a consumer-free chain) => the 8-gather chain ~11.3us is
    a hardware floor; 1024 rows / 128 offsets-per-instruction forces 8.
  - Final DMA completion leg is SIZE-dependent: a 4KB [128, 8] out settles
    in ~1.7-2.4us; shrinking it to [128, 2-3] measured SLOWER (2.7-4.8us).
    Keep the out tile at [128, 8] even though only 3 columns are used.
  - gpsimd.dma_start for the labels load (SWDGE direct) measured +3us: it
    shares the SWDGE ring with the gathers. Keep labels on Sync HWDGE,
    issued BEFORE the x DMAs so its descriptors beat the 1MB x flood.
"""

import numpy as np

from concourse import bacc, bass, mybir
import concourse.tile as tile
from concourse.bass_utils import run_bass_kernel_spmd

B = 8192
C = 10000
D = 256
N_CORES = 8
BL = B // N_CORES  # rows per core
P = 128            # SBUF partitions
G = BL // P        # row groups per core (8)

_CLIP_LO = 1e-12

_nc_cache = None


def _build():
    global _nc_cache
    if _nc_cache is not None:
        return _nc_cache

    nc = bacc.Bacc()
    x_l = nc.dram_tensor("x_local", [BL, D], mybir.dt.float32, kind="ExternalInput")
    lab_l = nc.dram_tensor("labels_local", [BL], mybir.dt.int32, kind="ExternalInput")
    cen = nc.dram_tensor("centers", [C, D], mybir.dt.float32, kind="ExternalInput")
    out = nc.dram_tensor("partials", [P, G], mybir.dt.float32, kind="ExternalOutput")

    with tile.TileContext(nc) as tc:
        with (
            tc.tile_pool(name="big", bufs=1) as big,
            # gather dests get all 8 slots: late gathers never carry a
            # slot-release wait, keeping the SWDGE chain free of EVSEM stalls
            tc.tile_pool(name="ctp", bufs=G) as ctp,
        ):
            lt = big.tile([P, G], mybir.dt.int32)
            xt = big.tile([P, G, D], mybir.dt.float32)
            df = big.tile([P, G * D], mybir.dt.float32)
            sq = big.tile([P, G * D], mybir.dt.float32)
            acc = big.tile([P, G], mybir.dt.float32)

            # labels first: the gather chain hangs off this DMA
            nc.sync.dma_start(out=lt[:], in_=lab_l[:].rearrange("(p g) -> p g", g=G))
            x_ap = x_l[:].rearrange("(p g) d -> p g d", g=G)
            nc.sync.dma_start(out=xt[:, 0:G // 2, :], in_=x_ap[:, 0:G // 2, :])
            nc.sync.dma_start(out=xt[:, G // 2:, :], in_=x_ap[:, G // 2:, :])

            for g in range(G):
                ct = ctp.tile([P, D], mybir.dt.float32, tag="ct")
                nc.gpsimd.indirect_dma_start(
                    out=ct[:],
                    out_offset=None,
                    in_=cen[:],
                    in_offset=bass.IndirectOffsetOnAxis(ap=lt[:, g:g + 1], axis=0),
                )
                nc.vector.tensor_sub(
                    out=df[:, g * D:(g + 1) * D], in0=xt[:, g, :], in1=ct[:]
                )
                if g == 3:
                    nc.scalar.activation(
                        out=sq[:, 0:4 * D],
                        in_=df[:, 0:4 * D],
                        func=mybir.ActivationFunctionType.Square,
                        accum_out=acc[:, 0:1],
                    )
                elif g == 6:
                    nc.scalar.activation(
                        out=sq[:, 4 * D:7 * D],
                        in_=df[:, 4 * D:7 * D],
                        func=mybir.ActivationFunctionType.Square,
                        accum_out=acc[:, 1:2],
                    )
            nc.scalar.activation(
                out=sq[:, 7 * D:8 * D],
                in_=df[:, 7 * D:8 * D],
                func=mybir.ActivationFunctionType.Square,
                accum_out=acc[:, 2:3],
            )
            nc.sync.dma_start(out=out[:], in_=acc[:])

    nc.finalize()
    _nc_cache = nc
    return nc


def _run(x, labels, centers, **spmd_kwargs):
    nc = _build()
    x = np.ascontiguousarray(np.asarray(x), dtype=np.float32)
    labels = np.ascontiguousarray(np.asarray(labels)).astype(np.int32)
    centers = np.ascontiguousarray(np.asarray(centers), dtype=np.float32)

    in_maps = []
    for c in range(N_CORES):
        sl = slice(c * BL, (c + 1) * BL)
        in_maps.append(
            {
                "x_local": x[sl],
                "labels_local": labels[sl],
                "centers": centers,
            }
        )
    res = run_bass_kernel_spmd(nc, in_maps, list(range(N_CORES)), **spmd_kwargs)
    partials = np.stack([r["partials"] for r in res.results])  # [8, P, G]
    # only accumulator columns 0..2 are written; the rest is uninitialized
    loss = (
        partials[:, :, :3].astype(np.float64).sum() + B * (C - 1) * _CLIP_LO
    ) / B
    return np.asarray(loss, dtype=np.float32), res


def kernel(x, labels, centers):
    loss, _ = _run(x, labels, centers)
    return loss


# revision 4
# speedup vs baseline: 1.1652x; 1.0389x over previous
"""CenterLoss kernel for Trainium2 (8 NeuronCores, SPMD data-parallel).

Reference computes
    distmat[b,c] = ||x_b||^2 + ||c_c||^2 - 2<x_b, c_c>          [B, C]
    loss = sum(clip(distmat * onehot(labels), 1e-12, 1e12)) / B

Only distmat[b, labels[b]] survives the mask; each of the B*(C-1) masked
zeros becomes exactly 1e-12 under the clip. So instead of the [8192, 10000]
distmat (42 GFLOP), each core gathers its rows' centers with indirect DMA
and computes per-row squared distances; the host adds the closed-form
constant B*(C-1)*1e-12 and divides by B. Every real row distance is a
chi^2-like sum of 256 squared normals (~512 >> 1e-12), so the per-row clip
is a provable no-op and only partial SUMS leave the device.

Sharding: batch split 8 ways (1024 rows/core), centers replicated.

Per-core structure (all stock ISA):
  - labels DMA'd first on Sync as [128, 8] (row p*8+g at [p, g]).
  - x as two linear 512KB DMAs into [128, 8, 256] (partition p holds rows
    8p..8p+7 contiguously -> perfectly sequential HBM reads).
  - 8 indirect row-gathers (SWDGE) of 128 rows each, offset APs are single
    column slices of the labels tile.
  - DVE subtracts pipeline behind the gathers into one [128, 2048] diff
    tile; ACT Square runs as 3 accum chunks (groups 0-3, 4-6, 7) so the
    post-last-gather tail is only sub + one 256-col ACT + ACCUM read.
  - acc [128, 8] DMA'd out (cols 0..2 meaningful); host sums in f64.

Metric note: gauge's "exec time" = (last trace event) - (first non-overhead
instruction) = first framework const-AP MEMSET (~5.9us into the raw trace).
The early preamble (sem-table init, TENSOR_LOADs) is NOT counted, but the
~7us Bacc epilogue IS: after the final barrier each engine serially walks a
~constant ~60-entry event/semaphore clear list at ~115ns/entry regardless
of kernel size (measured identical for a near-empty kernel), then waits a
final DMA-side reset. Untouchable without breaking re-execution.

Measured timing model (exec ~27.5us, vs 28.6us session-start baseline):
  ~6.1-6.8  framework preamble (sem-table init, engine barriers) — FIXED,
            independent of kernel instruction count.
  +0.7      labels DMA issue (Sync HWDGE) + 1.7us land latency
            (DGE delay 650ns + SEM_PROP 900ns).
  ~9.5-20.7 gather chain: 8 x DMA_INDIRECT at ~1410ns pitch.
  +1.8      last gather data lands (descriptor drain + sem propagation)
  +1.2      sub7 + 256-col Square/accum + ACCUM read
  +0.6+1.7  out DMA issue + completion leg; +~1.9us profiler tail.

Hard-won HW constraints baked in (this runtime rejects/crashes or
silently corrupts otherwise):
  - Use Bacc, and finalize() before run: TRN2 codegen allows ONE sync-wait
    per instruction; Bacc's generate_event_semaphores splits multi-waits.
  - Stock instructions only: custom "Ant" ISA ops (tensor_tensor_reduce,
    dma_gather, ...) kill the exec unit (NRT_EXEC_UNIT_UNRECOVERABLE).
  - No in-place DVE ops (out aliasing an input) — same crash.
  - indirect_dma_start: offset AP must be a [128, 1] column (one offset per
    partition). Multi-column offset APs are BROKEN in the SWDGE ucode
    (probed: first descriptor reuses the previous gather's stale offset
    address, in-partition descriptors re-read one column, partition
    advance resets to column 0, some descriptors never fire,
    nondeterministic across cores). Offsets must be in SBUF (walrus
    rejects DRAM: "Vector-dynamic-offsets location must be SB").
  - DMA_INDIRECT costs 994ns fixed SWDGE overhead + 0.34ns/descriptor,
    pitch ~1410ns back-to-back, independent of Tile framework glue
    (probed with a consumer-free chain) => the 8-gather chain ~11.3us is
    a hardware floor; 1024 rows / 128 offsets-per-instruction forces 8.
  - Final DMA completion leg is SIZE-dependent: a 4KB [128, 8] out settles
    in ~1.7-2.4us; shrinking it to [128, 2-3] measured SLOWER (2.7-4.8us).
    Keep the out tile at [128, 8] even though only 3 columns are used.
  - gpsimd.dma_start for the labels load (SWDGE direct) measured +3us: it
    shares the SWDGE ring with the gathers. Keep labels on Sync HWDGE,
    issued BEFORE the x DMAs so its descriptors beat the 1MB x flood.
"""

import numpy as np

from concourse import bacc, bass, mybir
import concourse.tile as tile
from concourse.bass_utils import run_bass_kernel_spmd

B = 8192
C = 10000
D = 256
N_CORES = 8
BL = B // N_CORES  # rows per core
P = 128            # SBUF partitions
G = BL // P        # row groups per core (8)

_CLIP_LO = 1e-12

_nc_cache = None


def _build():
    global _nc_cache
    if _nc_cache is not None:
        return _nc_cache

    nc = bacc.Bacc()
    x_l = nc.dram_tensor("x_local", [BL, D], mybir.dt.float32, kind="ExternalInput")
    lab_l = nc.dram_tensor("labels_local", [BL], mybir.dt.int32, kind="ExternalInput")
    cen = nc.dram_tensor("centers", [C, D], mybir.dt.float32, kind="ExternalInput")
    out = nc.dram_tensor("partials", [P, G], mybir.dt.float32, kind="ExternalOutput")

    with tile.TileContext(nc) as tc:
        with (
            tc.tile_pool(name="big", bufs=1) as big,
            # gather dests get all 8 slots: late gathers never carry a
            # slot-release wait, keeping the SWDGE chain free of EVSEM stalls
            tc.tile_pool(name="ctp", bufs=G) as ctp,
        ):
            lt = big.tile([P, G], mybir.dt.int32)
            xt = big.tile([P, G, D], mybir.dt.float32)
            df = big.tile([P, G * D], mybir.dt.float32)
            sq = big.tile([P, G * D], mybir.dt.float32)
            acc = big.tile([P, G], mybir.dt.float32)

            # labels first: the gather chain hangs off this DMA
            nc.sync.dma_start(out=lt[:], in_=lab_l[:].rearrange("(p g) -> p g", g=G))
            # x after labels on Sync: the labels descriptors (which gate the
            # whole gather chain) enter the DMA engines before the 1MB x
            # flood. (x-via-Scalar was tried: won one A/B, lost the next —
            # inside run-to-run noise; Sync kept for the stronger evidence.)
            x_ap = x_l[:].rearrange("(p g) d -> p g d", g=G)
            nc.sync.dma_start(out=xt[:, 0:G // 2, :], in_=x_ap[:, 0:G // 2, :])
            nc.sync.dma_start(out=xt[:, G // 2:, :], in_=x_ap[:, G // 2:, :])

            for g in range(G):
                ct = ctp.tile([P, D], mybir.dt.float32, tag="ct")
                nc.gpsimd.indirect_dma_start(
                    out=ct[:],
                    out_offset=None,
                    in_=cen[:],
                    in_offset=bass.IndirectOffsetOnAxis(ap=lt[:, g:g + 1], axis=0),
                )
                nc.vector.tensor_sub(
                    out=df[:, g * D:(g + 1) * D], in0=xt[:, g, :], in1=ct[:]
                )
                if g == 3:
                    nc.scalar.activation(
                        out=sq[:, 0:4 * D],
                        in_=df[:, 0:4 * D],
                        func=mybir.ActivationFunctionType.Square,
                        accum_out=acc[:, 0:1],
                    )
                elif g == 6:
                    nc.scalar.activation(
                        out=sq[:, 4 * D:7 * D],
                        in_=df[:, 4 * D:7 * D],
                        func=mybir.ActivationFunctionType.Square,
                        accum_out=acc[:, 1:2],
                    )
            nc.scalar.activation(
                out=sq[:, 7 * D:8 * D],
                in_=df[:, 7 * D:8 * D],
                func=mybir.ActivationFunctionType.Square,
                accum_out=acc[:, 2:3],
            )
            nc.sync.dma_start(out=out[:], in_=acc[:])

    nc.finalize()
    _nc_cache = nc
    return nc


def _run(x, labels, centers, **spmd_kwargs):
    nc = _build()
    x = np.ascontiguousarray(np.asarray(x), dtype=np.float32)
    labels = np.ascontiguousarray(np.asarray(labels)).astype(np.int32)
    centers = np.ascontiguousarray(np.asarray(centers), dtype=np.float32)

    in_maps = []
    for c in range(N_CORES):
        sl = slice(c * BL, (c + 1) * BL)
        in_maps.append(
            {
                "x_local": x[sl],
                "labels_local": labels[sl],
                "centers": centers,
            }
        )
    res = run_bass_kernel_spmd(nc, in_maps, list(range(N_CORES)), **spmd_kwargs)
    partials = np.stack([r["partials"] for r in res.results])  # [8, P, G]
    # only accumulator columns 0..2 are written; the rest is uninitialized
    loss = (
        partials[:, :, :3].astype(np.float64).sum() + B * (C - 1) * _CLIP_LO
    ) / B
    return np.asarray(loss, dtype=np.float32), res


def kernel(x, labels, centers):
    loss, _ = _run(x, labels, centers)
    return loss


# revision 5
# speedup vs baseline: 1.1816x; 1.0141x over previous
"""CenterLoss kernel for Trainium2 (8 NeuronCores, SPMD data-parallel).

Reference computes
    distmat[b,c] = ||x_b||^2 + ||c_c||^2 - 2<x_b, c_c>          [B, C]
    loss = sum(clip(distmat * onehot(labels), 1e-12, 1e12)) / B

Only distmat[b, labels[b]] survives the mask; each of the B*(C-1) masked
zeros becomes exactly 1e-12 under the clip. So instead of the [8192, 10000]
distmat (42 GFLOP), each core gathers its rows' centers with indirect DMA
and computes per-row squared distances; the host adds the closed-form
constant B*(C-1)*1e-12 and divides by B. Every real row distance is a
chi^2-like sum of 256 squared normals (~512 >> 1e-12), so the per-row clip
is a provable no-op and only partial SUMS leave the device.

Sharding: batch split 8 ways (1024 rows/core), centers replicated.

Per-core structure (all stock ISA):
  - labels DMA'd first on Sync as [128, 8] (row p*8+g at [p, g]).
  - x as two linear 512KB DMAs into [128, 8, 256] (partition p holds rows
    8p..8p+7 contiguously -> perfectly sequential HBM reads).
  - 8 indirect row-gathers (SWDGE) of 128 rows each, offset APs are single
    column slices of the labels tile.
  - DVE subtracts pipeline behind the gathers into one [128, 2048] diff
    tile; ACT Square runs as accum chunks (groups 0-3, 4-6) and group 7's
    256 cols are SPLIT: Scalar squares cols 0-127 while DVE mul+reduces
    cols 128-255 in parallel — the post-last-gather tail is 2.93us vs
    3.27us for the serial sub+ACT+READ chain (trace-verified).
  - acc [128, 8] DMA'd out (cols 0..3 meaningful); host sums in f64.

Metric note: gauge's "exec time" = (last trace event) - (first non-overhead
instruction) = first framework const-AP MEMSET (~5.9us into the raw trace).
The early preamble (sem-table init, TENSOR_LOADs) is NOT counted, but the
~7us Bacc epilogue IS: after the final barrier each engine serially walks a
~constant ~60-entry event/semaphore clear list at ~115ns/entry regardless
of kernel size (measured identical for a near-empty kernel), then waits a
final DMA-side reset. Untouchable without breaking re-execution.

Measured timing model (exec ~27.5us, vs 28.6us session-start baseline):
  ~6.1-6.8  framework preamble (sem-table init, engine barriers) — FIXED,
            independent of kernel instruction count.
  +0.7      labels DMA issue (Sync HWDGE) + 1.7us land latency
            (DGE delay 650ns + SEM_PROP 900ns).
  ~9.5-20.7 gather chain: 8 x DMA_INDIRECT at ~1410ns pitch.
  +1.8      last gather data lands (descriptor drain + sem propagation)
  +1.2      sub7 + 256-col Square/accum + ACCUM read
  +0.6+1.7  out DMA issue + completion leg; +~1.9us profiler tail.

Hard-won HW constraints baked in (this runtime rejects/crashes or
silently corrupts otherwise):
  - Use Bacc, and finalize() before run: TRN2 codegen allows ONE sync-wait
    per instruction; Bacc's generate_event_semaphores splits multi-waits.
  - Stock instructions only: custom "Ant" ISA ops (tensor_tensor_reduce,
    dma_gather, ...) kill the exec unit (NRT_EXEC_UNIT_UNRECOVERABLE).
  - No in-place DVE ops (out aliasing an input) — same crash.
  - indirect_dma_start: offset AP must be a [128, 1] column (one offset per
    partition). Multi-column offset APs are BROKEN in the SWDGE ucode
    (probed: first descriptor reuses the previous gather's stale offset
    address, in-partition descriptors re-read one column, partition
    advance resets to column 0, some descriptors never fire,
    nondeterministic across cores). Offsets must be in SBUF (walrus
    rejects DRAM: "Vector-dynamic-offsets location must be SB").
  - DMA_INDIRECT costs 994ns fixed SWDGE overhead + 0.34ns/descriptor,
    pitch ~1410ns back-to-back, independent of Tile framework glue
    (probed with a consumer-free chain) => the 8-gather chain ~11.3us is
    a hardware floor; 1024 rows / 128 offsets-per-instruction forces 8.
  - Final DMA completion leg is SIZE-dependent: a 4KB [128, 8] out settles
    in ~1.7-2.4us; shrinking it to [128, 2-3] measured SLOWER (2.7-4.8us).
    Keep the out tile at [128, 8] even though only 3 columns are used.
  - gpsimd.dma_start for the labels load (SWDGE direct) measured +3us: it
    shares the SWDGE ring with the gathers. Keep labels on Sync HWDGE,
    issued BEFORE the x DMAs so its descriptors beat the 1MB x flood.
"""

import numpy as np

from concourse import bacc, bass, mybir
import concourse.tile as tile
from concourse.bass_utils import run_bass_kernel_spmd

B = 8192
C = 10000
D = 256
N_CORES = 8
BL = B // N_CORES  # rows per core
P = 128            # SBUF partitions
G = BL // P        # row groups per core (8)

_CLIP_LO = 1e-12

_nc_cache = None


def _build():
    global _nc_cache
    if _nc_cache is not None:
        return _nc_cache

    nc = bacc.Bacc()
    x_l = nc.dram_tensor("x_local", [BL, D], mybir.dt.float32, kind="ExternalInput")
    lab_l = nc.dram_tensor("labels_local", [BL], mybir.dt.int32, kind="ExternalInput")
    cen = nc.dram_tensor("centers", [C, D], mybir.dt.float32, kind="ExternalInput")
    out = nc.dram_tensor("partials", [P, G], mybir.dt.float32, kind="ExternalOutput")

    with tile.TileContext(nc) as tc:
        with (
            tc.tile_pool(name="big", bufs=1) as big,
            # gather dests get all 8 slots: late gathers never carry a
            # slot-release wait, keeping the SWDGE chain free of EVSEM stalls
            tc.tile_pool(name="ctp", bufs=G) as ctp,
        ):
            lt = big.tile([P, G], mybir.dt.int32)
            xt = big.tile([P, G, D], mybir.dt.float32)
            df = big.tile([P, G * D], mybir.dt.float32)
            sq = big.tile([P, G * D], mybir.dt.float32)
            acc = big.tile([P, G], mybir.dt.float32)

            # labels first: the gather chain hangs off this DMA
            nc.sync.dma_start(out=lt[:], in_=lab_l[:].rearrange("(p g) -> p g", g=G))
            # x after labels on Sync: the labels descriptors (which gate the
            # whole gather chain) enter the DMA engines before the 1MB x
            # flood. (x-via-Scalar was tried: won one A/B, lost the next —
            # inside run-to-run noise; Sync kept for the stronger evidence.)
            x_ap = x_l[:].rearrange("(p g) d -> p g d", g=G)
            nc.sync.dma_start(out=xt[:, 0:G // 2, :], in_=x_ap[:, 0:G // 2, :])
            nc.sync.dma_start(out=xt[:, G // 2:, :], in_=x_ap[:, G // 2:, :])

            for g in range(G):
                ct = ctp.tile([P, D], mybir.dt.float32, tag="ct")
                nc.gpsimd.indirect_dma_start(
                    out=ct[:],
                    out_offset=None,
                    in_=cen[:],
                    in_offset=bass.IndirectOffsetOnAxis(ap=lt[:, g:g + 1], axis=0),
                )
                if g < G - 1:
                    nc.vector.tensor_sub(
                        out=df[:, g * D:(g + 1) * D], in0=xt[:, g, :], in1=ct[:]
                    )
                else:
                    nc.vector.tensor_sub(
                        out=df[:, g * D:g * D + D // 2],
                        in0=xt[:, g, 0:D // 2], in1=ct[:, 0:D // 2],
                    )
                    nc.vector.tensor_sub(
                        out=df[:, g * D + D // 2:(g + 1) * D],
                        in0=xt[:, g, D // 2:], in1=ct[:, D // 2:],
                    )
                if g == 3:
                    nc.scalar.activation(
                        out=sq[:, 0:4 * D],
                        in_=df[:, 0:4 * D],
                        func=mybir.ActivationFunctionType.Square,
                        accum_out=acc[:, 0:1],
                    )
                elif g == 6:
                    nc.scalar.activation(
                        out=sq[:, 4 * D:7 * D],
                        in_=df[:, 4 * D:7 * D],
                        func=mybir.ActivationFunctionType.Square,
                        accum_out=acc[:, 1:2],
                    )
            # group 7 tail split across engines: Scalar squares the first
            # 128 cols while DVE mul+reduces the last 128 — both finish
            # ~0.94us after the last gather lands instead of the 1.21us
            # serial sub+ACT+READ chain
            h = 7 * D + D // 2
            nc.scalar.activation(
                out=sq[:, 7 * D:h],
                in_=df[:, 7 * D:h],
                func=mybir.ActivationFunctionType.Square,
                accum_out=acc[:, 2:3],
            )
            nc.vector.tensor_mul(out=sq[:, h:8 * D], in0=df[:, h:8 * D], in1=df[:, h:8 * D])
            nc.vector.tensor_reduce(
                out=acc[:, 3:4],
                in_=sq[:, h:8 * D],
                axis=mybir.AxisListType.X,
                op=mybir.AluOpType.add,
            )
            nc.sync.dma_start(out=out[:], in_=acc[:])

    nc.finalize()
    _nc_cache = nc
    return nc


def _run(x, labels, centers, **spmd_kwargs):
    nc = _build()
    x = np.ascontiguousarray(np.asarray(x), dtype=np.float32)
    labels = np.ascontiguousarray(np.asarray(labels)).astype(np.int32)
    centers = np.ascontiguousarray(np.asarray(centers), dtype=np.float32)

    in_maps = []
    for c in range(N_CORES):
        sl = slice(c * BL, (c + 1) * BL)
        in_maps.append(
            {
                "x_local": x[sl],
                "labels_local": labels[sl],
                "centers": centers,
            }
        )
    res = run_bass_kernel_spmd(nc, in_maps, list(range(N_CORES)), **spmd_kwargs)
    partials = np.stack([r["partials"] for r in res.results])  # [8, P, G]
    # only accumulator columns 0..3 are written; the rest is uninitialized
    loss = (
        partials[:, :, :4].astype(np.float64).sum() + B * (C - 1) * _CLIP_LO
    ) / B
    return np.asarray(loss, dtype=np.float32), res


def kernel(x, labels, centers):
    loss, _ = _run(x, labels, centers)
    return loss
